# revision 17
# baseline (speedup 1.0000x reference)
"""Tensor-parallel GQA attention block (dense_transformer) on 8 TRN2 NeuronCores.

Sharding: tensor parallel across heads — core c owns q-heads 4c..4c+3 and
kv-head c (GQA groups intact). Each core AllGathers softmax-normalized
per-head attention outputs y (bf16, one collective per head per seq-pair,
8 total) and computes a 512-column slice of the output projection; the
host concatenates slices.

Device-side design (v2 — trimmed/causal-envelope attention):
  - all matmul operands "contraction dim on partitions": xT [DIM,S],
    wqkvT (q|k|v fused) [DIM,768], woT [DIM,512]; scores computed
    transposed (S^T = K-tile^T @ Q^T); V PE-transposed once to [s,hd].
  - PSUM tags: S (4 banks) = proj q0..q3 outputs / attn score ring,
    C (2) = proj k,v / attn psy, D (2) = V-transpose scratch / attn psd.
  - attention is trimmed to the 128-aligned causal envelope: per j-tile t
    and 512-wide i-half, only i >= 128t is computed (left-trim), so
    scores/PV/denominator matmuls and exp stream ~15% fewer elements and
    fully-masked halves are skipped. Diagonal chunks are right-aligned in
    their score-ring slot so the (u0,u1) pair is contiguous and exp'd in
    one ACT instruction; the causal triangle is a constant [128,128]
    -1e9 mask tensor_add'ed in-place in PSUM on the 128 diagonal columns
    only.
  - denominators: ones-vector matmuls accumulate row sums of exp(S^T)
    into psd (trimmed like psy); 1/D via fast custom-DVE reciprocal,
    broadcast on GpSimd, applied off the PE critical path. psy is
    evicted per-512-half as soon as its half finishes accumulating.
  - per-head AllGather (256KB in / 2MB out, 8 total) pipelines the
    collectives under proj(3)/attn(1)/outproj compute instead of two
    big 48us AllGathers on the tail.
  - proj chunk boundaries: per-bank PSUM eviction split across ACT and
    DVE engines (bf16 outputs), next chunk's k=0 matmuls ordered
    q0,q2,q1,q3,k,v to chase the eviction order; V-transposes of chunk
    ci are emitted inside proj(ci+1) so they never stall the PE. RoPE
    runs fully in bf16 (2x DVE rate).
  - ~24 dummy N=256 matmuls at kernel start warm the PE HAM clock gate
    during the initial DMA latency; proj(0) interleaves wqkv and x tile
    loads k-by-k so the first real matmul isn't starved.
  - compute dtype: bf16 matmul operands (fp32 PSUM accumulation), fp32
    softmax arithmetic, bf16 RoPE.
"""

import ml_dtypes
import numpy as np

import concourse.bass as bass
import concourse.mybir as mybir
import concourse.tile as tile
from concourse import bacc
from concourse.bass_utils import run_bass_kernel_spmd

F32 = mybir.dt.float32
BF16 = mybir.dt.bfloat16
AF = mybir.ActivationFunctionType

N_CORES = 8
DIM = 4096
S = 2048
HEAD_DIM = 128
N_HEADS = 32
N_KV = 8
HPC = N_HEADS // N_CORES        # q heads per core = 4
P = 128
SC = 512                        # seq chunk (free dim of most matmuls)
IC = 1024                       # attention i-chunk (2 seq chunks)
N_SCHUNK = S // SC              # 4
N_KTILE = DIM // P              # 32
N_STILE = S // P                # 16

SWAP16 = list(range(16, 32)) + list(range(16))   # per-quadrant 16-rotation


def build(debug_taps: bool = False):
    nc = bacc.Bacc(None, num_devices=N_CORES)

    xT = nc.declare_dram_parameter("xT", [DIM, S], BF16, isOutput=False)
    # fused qkv weights: [:, 0:512] q heads, [:, 512:640] k, [:, 640:768] v
    wqkvT = nc.declare_dram_parameter("wqkvT", [DIM, 768], BF16, isOutput=False)
    woT = nc.declare_dram_parameter("woT", [DIM, SC], BF16, isOutput=False)
    cosd = nc.declare_dram_parameter("cosd", [P, S], F32, isOutput=False)
    sins = nc.declare_dram_parameter("sins", [P, S], F32, isOutput=False)
    # constant causal triangle for a 128x128 diagonal block:
    # tri[j, i] = 0 if i >= j else -1e9
    trid = nc.declare_dram_parameter("trid", [P, P], F32, isOutput=False)
    out = nc.dram_tensor("out", [S, SC], F32, kind="ExternalOutput")

    taps = {}
    if debug_taps:
        taps["qt"] = nc.dram_tensor("qt", [P, HPC, S], F32, kind="ExternalOutput")
        taps["kt"] = nc.dram_tensor("kt", [P, S], F32, kind="ExternalOutput")
        taps["vv"] = nc.dram_tensor("vv", [P, N_STILE, HEAD_DIM], F32, kind="ExternalOutput")
        taps["dd"] = nc.dram_tensor("dd", [HPC, S], F32, kind="ExternalOutput")
        taps["yl"] = nc.dram_tensor("yl", [P, HPC, S], BF16, kind="ExternalOutput")

    with tile.TileContext(nc) as tc:
        # PSUM tags (8 banks total):
        #   S [P, 4*SC] f32 (4 banks): proj q0..q3 / attn score ring
        #   C [P, 2, SC] f32 (2 banks): proj {k,v} / attn psy
        #   D [P, 2, SC] f32 (2 banks): V-transpose scratch / attn psd
        ps = tc.alloc_tile_pool(name="ps", bufs=1, space="PSUM")
        const = tc.alloc_tile_pool(name="const", bufs=1)
        pw2 = tc.alloc_tile_pool(name="pw2", bufs=1, side="right")
        pw = tc.alloc_tile_pool(name="pw", bufs=1, side="right")
        obp = tc.alloc_tile_pool(name="obp", bufs=2)
        main = tc.alloc_tile_pool(name="main", bufs=1)
        stream = tc.alloc_tile_pool(name="stream", bufs=3)
        tmp = tc.alloc_tile_pool(name="tmp", bufs=2)
        dram = tc.alloc_tile_pool(name="dram", bufs=1, space="DRAM")

        # ---- constants + PE warmup ---------------------------------------
        warm = const.tile([P, 256], BF16)
        nc.vector.memset(warm[:], 0.0)
        pswarm = ps.tile([P, 2, SC], F32, tag="D", name="pswarm")
        for i in range(24):
            nc.tensor.matmul(pswarm[:, i % 2, 0:256], warm[:, 0:P], warm[:],
                             start=True, stop=True)

        ones_f = const.tile([P, P], F32)
        nc.vector.memset(ones_f[:], 1.0)
        ones = const.tile([P, P], BF16)
        nc.scalar.copy(ones[:], ones_f[:])
        ident = const.tile([P, P], F32)
        from concourse.masks import make_identity
        make_identity(nc, ident[:])
        tri_sb = const.tile([P, P], F32)
        nc.gpsimd.dma_start(tri_sb[:], trid[:])
        cos_sb = pw.tile([P, S], F32)
        sin_sb = pw.tile([P, S], F32)
        wqkv_sb = pw.tile([P, N_KTILE, 768], BF16)
        wo_sb = pw2.tile([P, N_KTILE, SC], BF16)

        kt_sb = main.tile([P, S], BF16)
        v_sb = main.tile([P, N_STILE, HEAD_DIM], BF16)
        qt_sb = main.tile([P, HPC, S], BF16)

        ybounce = [[
            dram.tile([P, IC], BF16, name=f"ybounce{cp}_{h}")
            for h in range(HPC)] for cp in range(2)
        ]
        ygather = [[
            dram.tile([N_CORES * P, IC], BF16, addr_space="Shared",
                      name=f"ygather{cp}_{h}")
            for h in range(HPC)] for cp in range(2)
        ]
        yg_tiles = {}
        raws = {}   # chunk ci -> dict of raw eviction tiles

        # ---- per-chunk segment emitters ----------------------------------
        def proj_mm(ci):
            """Emit projection matmuls + evictions + rope for chunk ci.
            V-transposes for chunk ci are NOT emitted here (see
            proj_vtrans) so they can't stall the next chunk's matmuls."""
            s_lo = ci * SC
            psq = ps.tile([P, 4 * SC], F32, tag="S", name=f"psq_{ci}")
            pskv = ps.tile([P, 2, SC], F32, tag="C", name=f"pskv_{ci}")
            for k in range(N_KTILE):
                if ci == 0:
                    nc.scalar.dma_start(wqkv_sb[:, k, :],
                                        wqkvT[k * P:(k + 1) * P, :])
                xs = stream.tile([P, SC], BF16, tag="xs", bufs=12, name=f"xs{ci}_{k}")
                nc.sync.dma_start(xs[:], xT[k * P:(k + 1) * P, s_lo:s_lo + SC])
                st = dict(start=(k == 0), stop=(k == N_KTILE - 1))
                # order q0,q2,q1,q3,k,v chases the split eviction order of
                # the previous chunk (ACT evicts S0,S1,C0; DVE S2,S3,C1)
                for h in (0, 2, 1, 3):
                    nc.tensor.matmul(
                        psq[:, h * SC:(h + 1) * SC],
                        wqkv_sb[:, k, h * P:(h + 1) * P], xs[:], **st
                    )
                nc.tensor.matmul(pskv[:, 0, :], wqkv_sb[:, k, 512:640], xs[:], **st)
                nc.tensor.matmul(pskv[:, 1, :], wqkv_sb[:, k, 640:768], xs[:], **st)

            # per-bank eviction split across ACT and DVE (f32 — the DVE
            # stream_shuffle used by rope does not handle bf16)
            rw = {}
            for name_, src, eng in (
                ("q0", psq[:, 0 * SC:1 * SC], nc.scalar),
                ("q2", psq[:, 2 * SC:3 * SC], nc.vector),
                ("q1", psq[:, 1 * SC:2 * SC], nc.scalar),
                ("q3", psq[:, 3 * SC:4 * SC], nc.vector),
                ("k", pskv[:, 0, :], nc.scalar),
                ("v", pskv[:, 1, :], nc.vector),
            ):
                raw = tmp.tile([P, SC], F32, tag=f"raw{name_}", bufs=2,
                               name=f"r{name_}_{ci}")
                if eng is nc.scalar:
                    nc.scalar.copy(raw[:], src)
                else:
                    nc.vector.tensor_copy(raw[:], src)
                rw[name_] = raw
            raws[ci] = rw

            # rope for q0..q3 and k (f32 math, bf16 destination)
            for h, name_ in enumerate(("q0", "q1", "q2", "q3", "k")):
                raw = rw[name_]
                dst = kt_sb[:, s_lo:s_lo + SC] if h == HPC \
                    else qt_sb[:, h, s_lo:s_lo + SC]
                qc = tmp.tile([P, SC], F32, tag="rqc", bufs=1, name=f"rq{ci}_{h}")
                nc.vector.tensor_mul(qc[:], raw[:], cos_sb[:, s_lo:s_lo + SC])
                qsw = tmp.tile([P, SC], F32, tag="rqs", bufs=1, name=f"rs{ci}_{h}")
                nc.vector.stream_shuffle(qsw[:], raw[:], SWAP16)
                nc.vector.tensor_mul(qsw[:], qsw[:], sin_sb[:, s_lo:s_lo + SC])
                nc.vector.tensor_add(dst, qc[:], qsw[:])

        def proj_vtrans(ci):
            """PE-transpose chunk ci's V [hd, 512] -> four [s,hd] tiles."""
            vt = raws[ci]["v"]
            pstT = ps.tile([P, 2, SC], F32, tag="D", name=f"pst{ci}")
            for q in range(SC // P):
                nc.tensor.transpose(
                    pstT[:, q % 2, 0:P], vt[:, q * P:(q + 1) * P], ident[:]
                )
                nc.vector.tensor_copy(v_sb[:, ci * 4 + q, :], pstT[:, q % 2, 0:P])

        def attn(cp, extra=None):
            """Software-pipelined: psy/psd for tile t-1 are emitted after
            scores/mask/exp of tile t, so the in-order PE queue always has
            independent score matmuls to run while ACT computes exp. The
            per-head normalization epilogue (DVE/GpSimd chain) is emitted
            during the NEXT head's first tiles so it never head-of-line
            blocks the DVE mask adds. `extra` is a list of emission
            closures sprinkled one-per-head (e.g. early yg loads)."""
            n_t = 8 * (cp + 1)
            pend_mm = None          # psy/psd emission for previous tile
            pend_copy = []          # delayed psy-half evictions
            pend_epi = None         # previous head's normalization chain

            def head(h):
                nonlocal pend_mm, pend_epi
                psy = ps.tile([P, 2, SC], F32, tag="C", name=f"psy{h}_{cp}")
                psd = ps.tile([1, IC], F32, tag="D", name=f"psd{h}_{cp}")
                Sr = ps.tile([P, 4 * SC], F32, tag="S", name=f"Sr{h}_{cp}")
                ysb = tmp.tile([P, IC], F32, tag="ysb", bufs=2, name=f"ysb{h}_{cp}")
                slot = 0
                for t in range(n_t):
                    u0_valid = t < 8 * cp + 4

                    def geom(u):
                        base = cp * IC + SC * u
                        trim = max(0, 128 * t - base)
                        return base, trim, SC - trim
                    if u0_valid:
                        if slot % 2:
                            slot = (slot + 1) % 4
                        s0 = slot
                        slot = (slot + 2) % 4
                        b0, tr0, w0 = geom(0)
                        b1, _, _ = geom(1)
                        c_lo = s0 * SC + (SC - w0)       # ring col of chunk start
                        nc.tensor.matmul(
                            Sr[:, c_lo:(s0 + 1) * SC],
                            kt_sb[:, t * P:(t + 1) * P],
                            qt_sb[:, h, b0 + tr0:b0 + SC],
                            start=True, stop=True,
                        )
                        nc.tensor.matmul(
                            Sr[:, (s0 + 1) * SC:(s0 + 2) * SC],
                            kt_sb[:, t * P:(t + 1) * P],
                            qt_sb[:, h, b1:b1 + SC],
                            start=True, stop=True,
                        )
                        if t >= 8 * cp:   # u0 is the diagonal chunk
                            nc.vector.tensor_add(
                                Sr[:, c_lo:c_lo + P], Sr[:, c_lo:c_lo + P],
                                tri_sb[:],
                            )
                        pt = tmp.tile([P, IC], BF16, tag="pt", bufs=4,
                                      name=f"pt{h}_{cp}_{t}")
                        p_lo = SC - w0
                        nc.scalar.activation(
                            pt[:, p_lo:IC], Sr[:, c_lo:(s0 + 2) * SC], AF.Exp
                        )
                        mm_chunks = [(0, tr0, w0, p_lo), (1, 0, SC, SC)]
                    else:
                        s0 = slot
                        slot = (slot + 1) % 4
                        b1, tr1, w1 = geom(1)
                        c_lo = s0 * SC + (SC - w1)
                        nc.tensor.matmul(
                            Sr[:, c_lo:(s0 + 1) * SC],
                            kt_sb[:, t * P:(t + 1) * P],
                            qt_sb[:, h, b1 + tr1:b1 + SC],
                            start=True, stop=True,
                        )
                        nc.vector.tensor_add(
                            Sr[:, c_lo:c_lo + P], Sr[:, c_lo:c_lo + P],
                            tri_sb[:],
                        )
                        pt = tmp.tile([P, IC], BF16, tag="pt", bufs=4,
                                      name=f"pt{h}_{cp}_{t}")
                        p_lo = SC - w1
                        nc.scalar.activation(
                            pt[:, p_lo:SC], Sr[:, c_lo:(s0 + 1) * SC], AF.Exp
                        )
                        mm_chunks = [(1, tr1, w1, p_lo)]

                    if t == 0:
                        while pend_copy:
                            pend_copy.pop(0)()
                    if t == 1 and pend_epi:
                        # previous head's normalization: must precede this
                        # head's first psy/psd emission (bank reuse order)
                        pend_epi()
                        pend_epi = None
                    if pend_mm:
                        pend_mm()

                    def mk_mm(t, mm_chunks, pt):
                        def emit():
                            for u, trim, w, plo in mm_chunks:
                                lo = SC * u + trim
                                st = dict(start=(t == 0),
                                          stop=(t == 8 * cp + 4 * u + 3))
                                nc.tensor.matmul(psy[:, u, trim:SC],
                                                 v_sb[:, t, :],
                                                 pt[:, plo:plo + w], **st)
                                nc.tensor.matmul(psd[:, lo:lo + w],
                                                 ones[:, 0:1],
                                                 pt[:, plo:plo + w], **st)
                                if st["stop"]:
                                    def cp_(u=u):
                                        nc.vector.tensor_copy(
                                            ysb[:, u * SC:(u + 1) * SC],
                                            psy[:, u, :])
                                    pend_copy.append(cp_)
                        return emit
                    pend_mm = mk_mm(t, mm_chunks, pt)

                pend_mm()
                pend_mm = None

                def epi():
                    rc1 = tmp.tile([1, IC], F32, tag="rc1", bufs=1,
                                   name=f"rc1{h}_{cp}")
                    nc.vector.reciprocal_approx_fast(rc1[:], psd[:])
                    rbb = tmp.tile([P, IC], F32, tag="rbb", bufs=1,
                                   name=f"rbb{h}_{cp}")
                    nc.gpsimd.partition_broadcast(rbb[:], rc1[:])
                    yp = tmp.tile([P, IC], BF16, tag="yp", name=f"yp{h}_{cp}")
                    nc.vector.tensor_mul(yp[:], ysb[:], rbb[:])
                    nc.gpsimd.dma_start(ybounce[cp][h][:], yp[:])
                    if debug_taps:
                        s_lo = cp * IC
                        dsb = tmp.tile([1, IC], F32, tag="dsb", bufs=1,
                                       name=f"dsb{h}_{cp}")
                        nc.vector.tensor_copy(dsb[:], psd[:])
                        nc.gpsimd.dma_start(taps["yl"][:, h, s_lo:s_lo + IC], yp[:])
                        nc.gpsimd.dma_start(taps["dd"][h:h + 1, s_lo:s_lo + IC], dsb[:])
                    nc.gpsimd.collective_compute(
                        "AllGather",
                        mybir.AluOpType.bypass,
                        replica_groups=[list(range(N_CORES))],
                        ins=[ybounce[cp][h][:]],
                        outs=[ygather[cp][h][:]],
                    )
                pend_epi = epi

            for h in range(HPC):
                head(h)
                if extra and h < len(extra):
                    extra[h]()
            while pend_copy:
                pend_copy.pop(0)()
            pend_epi()

        def load_yg(cp, hh, pool):
            if cp not in yg_tiles:
                yg_tiles[cp] = [None] * HPC
            yg_tiles[cp][hh] = pool.tile([P, N_CORES, IC], BF16,
                                         tag=f"yg{cp}_{hh}", name=f"yg{cp}_{hh}")
            nc.sync.dma_start(
                yg_tiles[cp][hh][:],
                ygather[cp][hh][:].rearrange("(c p) m -> p c m", p=P),
            )

        def outproj(ci):
            g_lo = ci * SC
            cp, u = ci // 2, ci % 2
            for st_i in range(4):
                pso = ps.tile(
                    [P, 2, SC], F32, tag=("C" if st_i % 2 == 0 else "D"),
                    name=f"pso{ci}_{st_i}",
                )
                for kt in range(N_KTILE):
                    src_t = yg_tiles[cp][kt % HPC]
                    nc.tensor.matmul(
                        pso[:, 0, :],
                        src_t[:, kt // HPC,
                              u * SC + st_i * P:u * SC + (st_i + 1) * P],
                        wo_sb[:, kt, :],
                        start=(kt == 0), stop=(kt == N_KTILE - 1),
                    )
                ob = obp.tile([P, SC], F32, tag="ob", name=f"ob{ci}_{st_i}")
                nc.scalar.copy(ob[:], pso[:, 0, :])
                nc.gpsimd.dma_start(
                    out[g_lo + st_i * P:g_lo + (st_i + 1) * P, :], ob[:]
                )

        # ---- software-pipelined emission ---------------------------------
        nc.gpsimd.dma_start(cos_sb[:], cosd[:])
        nc.gpsimd.dma_start(sin_sb[:], sins[:])
        proj_mm(0)
        proj_mm(1)
        proj_vtrans(0)
        proj_mm(2)
        proj_vtrans(1)
        attn(0)          # -> per-head AGs, pair 0
        nc.scalar.dma_start(wo_sb[:], woT.rearrange("(t p) m -> p t m", p=P))
        proj_mm(3)
        proj_vtrans(2)
        proj_vtrans(3)

        if debug_taps:
            nc.gpsimd.dma_start(taps["qt"][:], qt_sb[:])
            nc.gpsimd.dma_start(taps["kt"][:], kt_sb[:])
            nc.gpsimd.dma_start(taps["vv"][:], v_sb[:])

        pw.release()
        pyg = tc.alloc_tile_pool(name="pyg", bufs=1, side="right")
        # cp0's gathered-y loads stream in during attn(1)
        attn(1, extra=[(lambda hh=hh: load_yg(0, hh, pyg)) for hh in range(HPC)])
        for pool in (tmp, stream, main):
            pool.release()
        pyg2 = tc.alloc_tile_pool(name="pyg2", bufs=1, side="right")
        for hh in range(HPC):
            load_yg(1, hh, pyg2)
        outproj(0)
        outproj(1)
        outproj(2)
        outproj(3)

        for pool in (pyg2, pyg, pw2, dram, obp, const, ps):
            pool.release()

    nc.compile()
    return nc


# ---------------------------------------------------------------------------
# host-side prep / unshard
# ---------------------------------------------------------------------------

def _perm128():
    """head-dim permutation: pair i=(16q+j) -> even at 32q+j, odd at 32q+16+j."""
    order = np.empty(128, dtype=np.int64)
    for i in range(64):
        q, j = i // 16, i % 16
        order[32 * q + j] = 2 * i
        order[32 * q + 16 + j] = 2 * i + 1
    return order


def _host_prep(x, freqs_cis, wq, wk, wv, wo):
    order = _perm128()
    xT = np.ascontiguousarray(x[0].T)                       # [DIM, S]
    scale = np.float32(1.0 / np.sqrt(HEAD_DIM))

    cosT = np.ascontiguousarray(freqs_cis[:, :, 0].T)       # [64, S]
    sinT = np.ascontiguousarray(freqs_cis[:, :, 1].T)
    cosd = np.empty((P, S), dtype=np.float32)
    sins = np.empty((P, S), dtype=np.float32)
    for q in range(4):
        cosd[32 * q:32 * q + 16] = cosT[16 * q:16 * q + 16]
        cosd[32 * q + 16:32 * q + 32] = cosT[16 * q:16 * q + 16]
        sins[32 * q:32 * q + 16] = -sinT[16 * q:16 * q + 16]
        sins[32 * q + 16:32 * q + 32] = sinT[16 * q:16 * q + 16]

    ii = np.arange(P)[None, :]
    jj = np.arange(P)[:, None]
    trid = np.where(ii >= jj, np.float32(0.0), np.float32(-1e9))
    trid = np.ascontiguousarray(trid, dtype=np.float32)

    xT16 = xT.astype(ml_dtypes.bfloat16)
    in_maps = []
    for c in range(N_CORES):
        wq_c = wq[c * 512:(c + 1) * 512].reshape(HPC, 128, DIM)[:, order, :]
        wq_c = (wq_c.reshape(512, DIM) * scale).astype(np.float32)
        wk_c = wk[c * 128:(c + 1) * 128][order]
        wv_c = wv[c * 128:(c + 1) * 128]
        wqkv_c = np.concatenate([wq_c, wk_c, wv_c], axis=0)
        wo_c = wo[c * 512:(c + 1) * 512]
        in_maps.append({
            "xT": xT16,
            "wqkvT": np.ascontiguousarray(wqkv_c.T).astype(ml_dtypes.bfloat16),
            "woT": np.ascontiguousarray(wo_c.T).astype(ml_dtypes.bfloat16),
            "cosd": cosd,
            "sins": sins,
            "trid": trid,
        })
    return in_maps


_NC_CACHE = {}


def get_nc(debug_taps=False):
    key = bool(debug_taps)
    if key not in _NC_CACHE:
        _NC_CACHE[key] = build(debug_taps=key)
    return _NC_CACHE[key]


def kernel(x, freqs_cis, mask, wq, wk, wv, wo, _trace=False, _debug_taps=False,
           _warmup=False):
    in_maps = _host_prep(x, freqs_cis, wq, wk, wv, wo)
    nc = get_nc(_debug_taps)
    if _warmup:
        run_bass_kernel_spmd(
            nc, in_maps, core_ids=list(range(N_CORES)), trace=False
        )
    res = run_bass_kernel_spmd(
        nc, in_maps, core_ids=list(range(N_CORES)), trace=_trace
    )
    full = np.concatenate([res.results[c]["out"] for c in range(N_CORES)], axis=1)
    out = full.reshape(1, S, DIM).astype(np.float32)
    if _trace or _debug_taps:
        kernel.last_results = res
    return out
